# revision 11
# baseline (speedup 1.0000x reference)
# Multi-head causal attention (B=2, T=2048, D=1024, H=16, HS=64) on 8 TRN2 NeuronCores.
#
# Sharding: core c = (batch b = c//4, head-group g = c%4 -> heads 4g..4g+3).
# Host pre-transposes x (kernel input xT = x[b].T, cast f16) and slices
# w_qkv columns / w_out rows per core; each core computes a partial (T, D)
# output projection and the host sums the 4 partials per batch (+ b_out).
#
# On-device dataflow (fast path, biases == 0):
#   Q^T,K^T [hs, t] come out of the QKV projection (w stationary, x^T moving);
#   V is computed in natural [t, hs] layout with an extra ones-column.
#   Scores are built as S^T [k, t] blocks, exp'd on ACT (no max-subtraction:
#   scores bounded, fp32 exp safe), diag triangles masked on GpSimd.
#   PV runs in "o-orientation": stationary P^T blocks [128k x 128q], moving
#   vones [128k, 65] -> po [128 q-part, 65] accumulated over k-blocks (the
#   65th column is the softmax denominator l).  This streams only 65 columns
#   per k-block (vs 128 q-cols in the oT orientation), halving PV PE time.
#   Normalization: 1/l per-partition scalar (DVE reciprocal + tensor_scalar).
#   o [q, e] is then PE-transposed (identity rhs) to oT [e, t] feeding the
#   output projection as the stationary operand.
import math
import os
import sys

import numpy as np

for _p in ("/opt/trn_rl_repo",):
    if _p not in sys.path and os.path.isdir(_p):
        sys.path.insert(0, _p)

import concourse.bass as bass
import concourse.mybir as mybir
import concourse.tile as tile
from concourse import bacc
from concourse import bass_utils
from concourse import library_config

B, T, D = 2, 2048, 1024
H, HS = 16, 64
NCORES = 8
GROUPS = NCORES // B          # head-groups per batch = 4
HPC = H // GROUPS             # heads per core = 4
EC = HPC * HS                 # head-dim cols per section per core = 256
DC = D // 128                 # d-chunks = 8
TT = T // 128                 # t-tiles = 16
QS = 512                      # q-supertile
NQS = T // QS                 # 4
SCALE = 1.0 / math.sqrt(HS)

F32 = mybir.dt.float32
F16 = mybir.dt.float16
CDT = mybir.dt.float16        # compute dtype for matmul operands


def _quads(qs):
    # quad = list of (kb, col_off, q0, nq); diagonal blocks packed
    # contiguously so one exp covers only valid columns.
    qds = []
    for kq in range(qs * 2):
        qds.append([(kq * 2, 0, 0, 512), (kq * 2 + 1, 512, 0, 512)])
    d0 = qs * 4
    qds.append([(d0 + 0, 0, 0, 512), (d0 + 1, 512, 128, 384)])
    qds.append([(d0 + 2, 0, 256, 256), (d0 + 3, 256, 384, 128)])
    return qds


def _locate(kb, qb, qs):
    # (quad index, col offset) of score block (k-block kb, q-block qb) in the
    # round-qs quad tiles.
    if kb < 4 * qs:
        return kb // 2, (kb % 2) * 512 + 128 * qb
    j = kb - 4 * qs
    if j == 0:
        return 2 * qs, 128 * qb
    if j == 1:
        return 2 * qs, 512 + 128 * (qb - 1)
    if j == 2:
        return 2 * qs + 1, 128 * (qb - 2)
    return 2 * qs + 1, 256 + 128 * (qb - 3)


def _mha_fast_kernel(tc, outp, xT, wq, wo, mask, ident):
    nc = tc.nc
    EXP = mybir.ActivationFunctionType.Exp

    with (
        tc.tile_pool(name="singles", bufs=1) as singles,
        tc.tile_pool(name="acts", bufs=1) as acts,
        tc.tile_pool(name="pt", bufs=24) as ptp,
        tc.tile_pool(name="rl", bufs=4) as rlp,
        tc.tile_pool(name="ob", bufs=4) as obp,
        tc.tile_pool(name="psum", bufs=1, space="PSUM") as psa,
    ):
        xT_sb = singles.tile([128, DC, T], CDT)
        w_sb = singles.tile([128, DC, 3 * EC], CDT)
        wo_sb = singles.tile([128, EC // 128, D], CDT)
        mask_sb = singles.tile([128, 128], CDT)
        ident_sb = singles.tile([128, 128], CDT)

        xT_r = xT.rearrange("(c p) t -> p c t", p=128)
        wq_r = wq.rearrange("(c p) e -> p c e", p=128)

        # ---- input DMAs (HWDGE via sync; emission order = transfer order).
        # Slab-major x, q|k weight columns first, finely sliced at the start
        # so the ts=0 projection can begin ~3.8us in.
        for dp in range(4):
            nc.sync.dma_start(out=w_sb[:, 2 * dp:2 * dp + 2, 0:512],
                              in_=wq_r[:, 2 * dp:2 * dp + 2, 0:512])
            nc.sync.dma_start(out=xT_sb[:, 2 * dp:2 * dp + 2, 0:512],
                              in_=xT_r[:, 2 * dp:2 * dp + 2, 0:512])
        nc.sync.dma_start(out=w_sb[:, 0:4, 512:768], in_=wq_r[:, 0:4, 512:768])
        nc.sync.dma_start(out=w_sb[:, 4:8, 512:768], in_=wq_r[:, 4:8, 512:768])
        nc.sync.dma_start(out=mask_sb, in_=mask)
        nc.sync.dma_start(out=xT_sb[:, 0:4, 512:1024], in_=xT_r[:, 0:4, 512:1024])
        nc.sync.dma_start(out=xT_sb[:, 4:8, 512:1024], in_=xT_r[:, 4:8, 512:1024])
        nc.sync.dma_start(out=xT_sb[:, :, 1024:1536], in_=xT_r[:, :, 1024:1536])
        nc.sync.dma_start(out=ident_sb, in_=ident)
        nc.sync.dma_start(out=xT_sb[:, :, 1536:2048], in_=xT_r[:, :, 1536:2048])
        nc.sync.dma_start(out=wo_sb, in_=wo.rearrange("(c p) e -> p c e", p=128))
        nc.gpsimd.load_library(library_config.standard)

        qkT_sb = acts.tile([128, 2 * EC // 128, T], CDT)
        vones_sb = acts.tile([128, TT, HPC, HS + 1], CDT)
        o_sb = acts.tile([128, TT, EC], CDT)
        oT_sb = acts.tile([128, EC // 128, T], CDT)
        nc.vector.memset(vones_sb[:, :, :, HS:HS + 1], 1.0)

        # PSUM (8 banks): "s" scores 2x[128,1024]; "po" PV accumulators
        # 2x[128,288] (4 x 72-col chains per bank); "pj" 2x[128,512] shared by
        # projections / transposes / out-proj.
        def emit_qk_prologue():
            # ts=0 q,k projection, dc-major four-phase so each dc-pair's
            # matmuls run while the next x/w slices are still in flight.
            sA = psa.tile([128, 1024], F32, tag="s", bufs=2, name="sA")
            sB = psa.tile([128, 1024], F32, tag="s", bufs=2, name="sB")
            tgt = [sA[:, 0:512], sA[:, 512:1024], sB[:, 0:512], sB[:, 512:1024]]
            for phase in range(4):
                for et in range(4):
                    for dc in range(2 * phase, 2 * phase + 2):
                        nc.tensor.matmul(
                            tgt[et],
                            lhsT=w_sb[:, dc, et * 128:(et + 1) * 128],
                            rhs=xT_sb[:, dc, 0:512],
                            start=(dc == 0),
                            stop=(dc == DC - 1),
                        )
            nc.vector.tensor_copy(out=qkT_sb[:, 0:2, 0:512], in_=sA)
            nc.vector.tensor_copy(out=qkT_sb[:, 2:4, 0:512], in_=sB)

        def emit_qk(et, ts):
            ps = psa.tile([128, 512], F32, tag="pj", bufs=2, name="psqk")
            for dc in range(DC):
                nc.tensor.matmul(
                    ps,
                    lhsT=w_sb[:, dc, et * 128:(et + 1) * 128],
                    rhs=xT_sb[:, dc, ts * 512:(ts + 1) * 512],
                    start=(dc == 0),
                    stop=(dc == DC - 1),
                )
            nc.vector.tensor_copy(
                out=qkT_sb[:, et, ts * 512:(ts + 1) * 512], in_=ps)

        def emit_v2(tp):
            # V projection for tt pair (2tp, 2tp+1): two 8-dc chains packed in
            # one bank (single start on the first, single stop on the last).
            psv = psa.tile([128, 512], F32, tag="pj", bufs=2, name="psv")
            for i in range(2):
                ttc = 2 * tp + i
                for dc in range(DC):
                    nc.tensor.matmul(
                        psv[:, i * 256:(i + 1) * 256],
                        lhsT=xT_sb[:, dc, ttc * 128:(ttc + 1) * 128],
                        rhs=w_sb[:, dc, 2 * EC:3 * EC],
                        start=(i == 0 and dc == 0),
                        stop=(i == 1 and dc == DC - 1),
                    )
            nc.vector.tensor_copy(
                out=vones_sb[:, 2 * tp:2 * tp + 2, :, 0:HS],
                in_=psv.rearrange("p (t h s) -> p t h s", t=2, h=HPC),
            )

        def emit_quads(h, qs):
            # QK^T score blocks + exp + diag masking; returns the round's pT
            # tiles (one per quad).
            pb = 64 * (h % 2)
            qT = qkT_sb[pb:pb + 64, h // 2, :]
            kT = qkT_sb[pb:pb + 64, 2 + h // 2, :]
            pts = []
            for quad in _quads(qs):
                sps = psa.tile([128, 1024], F32, tag="s", bufs=2, name="sps")
                pT = ptp.tile([128, 1024], CDT, tag="pT", name="pT")
                for (kb, off, q0, nq) in quad:
                    nc.tensor.matmul(
                        sps[:, off:off + nq],
                        lhsT=kT[:, kb * 128:(kb + 1) * 128],
                        rhs=qT[:, qs * 512 + q0:(qs + 1) * 512],
                        start=True,
                        stop=True,
                    )
                w = max(off + nq for (kb, off, q0, nq) in quad)
                nc.scalar.activation(out=pT[:, 0:w], in_=sps[:, 0:w],
                                     func=EXP, scale=SCALE)
                for (kb, off, q0, nq) in quad:
                    if kb >= qs * 4:  # diagonal: mask leading 128-col triangle
                        nc.gpsimd.tensor_mul(
                            out=pT[:, off:off + 128],
                            in0=pT[:, off:off + 128],
                            in1=mask_sb,
                        )
                pts.append(pT)
            return pts

        def emit_chains(h, qs, pts, qbs=(0, 1, 2, 3)):
            # PV in o-orientation: po[q, 0:64] = sum_k P^T[k,q-blk] V[k,:],
            # po[q, 64] = l.  The 72-col chains share one PSUM bank; one
            # start (clears the bank) and one stop on the last matmul.
            po = psa.tile([128, 512], F32, tag="po", bufs=2, name="po")
            for i, qb in enumerate(qbs):
                Q = 4 * qs + qb
                for kb in range(Q + 1):
                    j, col = _locate(kb, qb, qs)
                    nc.tensor.matmul(
                        po[:, 72 * i:72 * i + 65],
                        lhsT=pts[j][:, col:col + 128],
                        rhs=vones_sb[:, kb, h, :],
                        start=(i == 0 and kb == 0),
                        stop=(i == len(qbs) - 1 and kb == Q),
                    )
            # epilogue: per-partition 1/l, fused into the PSUM->SBUF evac
            rl = rlp.tile([128, len(qbs)], F32, tag="rl")
            lcols = bass.AP(
                tensor=po.tensor, offset=po.offset + 64,
                ap=[list(po.ap[0]), [72, len(qbs)]],
            )
            nc.vector.reciprocal(out=rl, in_=lcols)
            for i, qb in enumerate(qbs):
                nc.vector.tensor_scalar_mul(
                    out=o_sb[:, 4 * qs + qb, h * 64:(h + 1) * 64],
                    in0=po[:, 72 * i:72 * i + 64],
                    scalar1=rl[:, i:i + 1],
                )

        def emit_transpose(tts, ec):
            # o [q, e] -> oT [e, t] for the given t-tiles, one ec block
            # (= head pair 2ec, 2ec+1); transposes packed in one bank.
            tp = psa.tile([128, 512], CDT, tag="pj", bufs=2, name="tp")
            for i, tt in enumerate(tts):
                nc.tensor.transpose(
                    tp[:, i * 128:(i + 1) * 128],
                    o_sb[:, tt, ec * 128:(ec + 1) * 128],
                    ident_sb,
                )
            nc.vector.tensor_copy(
                out=oT_sb[:, ec, tts[0] * 128:(tts[0] + len(tts)) * 128],
                in_=tp[:, 0:128 * len(tts)])

        def emit_outproj(tt, split_store=False):
            outsb = obp.tile([128, 1024], F16, tag="ob", name="outsb")
            for half in range(2):
                pr = psa.tile([128, 512], F32, tag="pj", bufs=2, name="pso")
                for ec in range(EC // 128):
                    nc.tensor.matmul(
                        pr,
                        lhsT=oT_sb[:, ec, tt * 128:(tt + 1) * 128],
                        rhs=wo_sb[:, ec, half * 512:(half + 1) * 512],
                        start=(ec == 0),
                        stop=(ec == EC // 128 - 1),
                    )
                nc.vector.tensor_copy(
                    out=outsb[:, half * 512:(half + 1) * 512], in_=pr)
                if split_store:
                    nc.sync.dma_start(
                        out=outp[tt * 128:(tt + 1) * 128,
                                 half * 512:(half + 1) * 512],
                        in_=outsb[:, half * 512:(half + 1) * 512])
            if not split_store:
                nc.sync.dma_start(out=outp[tt * 128:(tt + 1) * 128, :],
                                  in_=outsb)

        # ---- emission: flattened software pipeline over 16 head-rounds.
        # Score quads (+exp) run LAG head-rounds ahead of their PV chains so
        # ACT has lookahead in the exp-heavy late rounds.  Projections are
        # front-loaded; transposes/out-proj are deferred into the late
        # rounds, where the serial exp stream otherwise starves PE.
        LAG = 2
        emit_qk_prologue()
        emit_v2(0)
        emit_v2(1)

        fill = {i: [] for i in range(16)}

        def sched(items, lo, hi):
            n = hi - lo + 1
            for i, it in enumerate(items):
                fill[lo + (i * n) // len(items)].append(it)

        for qs in range(NQS - 1):
            items = [lambda et=et, qs=qs: emit_qk(et, qs + 1) for et in (0, 2, 1, 3)]
            items += [lambda tp=tp: emit_v2(tp) for tp in (2 * qs + 2, 2 * qs + 3)]
            sched(items, 3 * qs, 3 * qs + 2)          # ts1: 0-2, ts2: 3-5, ts3: 6-8
        sched([lambda ec=ec: emit_transpose([0, 1, 2, 3], ec) for ec in (0, 1)],
              8, 9)
        sched([lambda tt=tt: emit_outproj(tt) for tt in range(0, 4)], 10, 11)
        sched([lambda ec=ec: emit_transpose([4, 5, 6, 7], ec) for ec in (0, 1)],
              12, 12)
        sched([lambda tt=tt: emit_outproj(tt) for tt in range(4, 8)], 13, 14)
        sched([lambda ec=ec: emit_transpose([8, 9, 10, 11], ec) for ec in (0, 1)],
              15, 15)

        quads = {}
        for i in range(16):
            qs, h = divmod(i, 4)
            quads[i] = emit_quads(h, qs)
            if i - LAG >= 0:
                pqs, ph = divmod(i - LAG, 4)
                emit_chains(ph, pqs, quads.pop(i - LAG))
            for f in fill[i]:
                f()
        # drain: remaining chains interleaved with round-2 out-proj, the very
        # last head-round split by q-block pair so the final out-projections
        # overlap its PV.
        emit_chains(2, 3, quads[14])
        emit_outproj(8)
        emit_outproj(9)
        emit_chains(3, 3, quads[15], qbs=(0, 1))
        emit_outproj(10)
        emit_outproj(11)
        for ec in range(2):
            emit_transpose([12, 13], ec)
        emit_outproj(12)
        emit_outproj(13)
        emit_chains(3, 3, quads[15], qbs=(2, 3))
        for ec in range(2):
            emit_transpose([14, 15], ec)
        emit_outproj(14, split_store=True)
        emit_outproj(15, split_store=True)


def build_nc_fast():
    nc = bacc.Bacc("TRN2", target_bir_lowering=False, debug=False)
    xT = nc.dram_tensor("xT", (D, T), F16, kind="ExternalInput")
    wq = nc.dram_tensor("wq", (D, 3 * EC), F16, kind="ExternalInput")
    wo = nc.dram_tensor("wo", (EC, D), F16, kind="ExternalInput")
    mask = nc.dram_tensor("mask", (128, 128), F16, kind="ExternalInput")
    ident = nc.dram_tensor("ident", (128, 128), F16, kind="ExternalInput")
    outp = nc.dram_tensor("outp", (T, D), F16, kind="ExternalOutput")
    with tile.TileContext(nc) as tc:
        _mha_fast_kernel(tc, outp[:], xT[:], wq[:], wo[:], mask[:], ident[:])
    nc.compile()
    return nc


def make_in_maps_fast(x, w_qkv, w_out):
    import ml_dtypes
    f16 = np.float16
    i = np.arange(128)
    mask = (i[None, :] >= i[:, None]).astype(f16)     # keep q >= k
    ident = np.eye(128, dtype=f16)
    in_maps = []
    for c in range(NCORES):
        b, g = divmod(c, GROUPS)
        cs = slice(EC * g, EC * (g + 1))
        wq_c = np.ascontiguousarray(np.concatenate(
            [w_qkv[:, cs], w_qkv[:, D:][:, cs], w_qkv[:, 2 * D:][:, cs]],
            axis=1).astype(f16))
        in_maps.append({
            "xT": np.ascontiguousarray(x[b].T.astype(f16)),
            "wq": wq_c,
            "wo": np.ascontiguousarray(w_out[cs, :].astype(f16)),
            "mask": mask,
            "ident": ident,
        })
    return in_maps


_NC_CACHE = {}


def get_nc():
    if "fast" not in _NC_CACHE:
        _NC_CACHE["fast"] = build_nc_fast()
    return _NC_CACHE["fast"]


def run_on_hw(in_maps, **kwargs):
    nc = get_nc()
    return bass_utils.run_bass_kernel_spmd(
        nc, in_maps, core_ids=list(range(NCORES)), **kwargs
    )


def kernel(x, w_qkv, b_qkv, w_out, b_out):
    x = np.asarray(x, dtype=np.float32)
    w_qkv = np.asarray(w_qkv, dtype=np.float32)
    b_qkv = np.asarray(b_qkv, dtype=np.float32)
    w_out = np.asarray(w_out, dtype=np.float32)
    b_out = np.asarray(b_out, dtype=np.float32)

    if np.any(b_qkv):
        return _kernel_bias(x, w_qkv, b_qkv, w_out, b_out)

    in_maps = make_in_maps_fast(x, w_qkv, w_out)
    res = run_on_hw(in_maps)
    parts = [r["outp"].astype(np.float64) for r in res.results]
    out = np.stack([
        sum(parts[GROUPS * b:GROUPS * (b + 1)]) for b in range(B)
    ]).astype(np.float32)
    return out + b_out[None, None, :]


# ---------------------------------------------------------------------------
# Fallback path for nonzero b_qkv (not hit by the reference input
# distribution): the original bias-capable kernel.
# ---------------------------------------------------------------------------


def _mha_tile_kernel_bias(tc, outp, xT, wq, wo, bqk, bv, mask):
    nc = tc.nc
    EXP = mybir.ActivationFunctionType.Exp
    BDT = mybir.dt.bfloat16

    with (
        tc.tile_pool(name="singles", bufs=1) as singles,
        tc.tile_pool(name="acts", bufs=1) as acts,
        tc.tile_pool(name="pt", bufs=8) as ptp,
        tc.tile_pool(name="rl", bufs=6) as rlp,
        tc.tile_pool(name="ob", bufs=6) as obp,
        tc.tile_pool(name="psum", bufs=1, space="PSUM") as psa,
    ):
        xT_sb = singles.tile([128, DC, T], BDT)
        w_sb = singles.tile([128, DC, 3 * EC], BDT)
        xT_r = xT.rearrange("(c p) t -> p c t", p=128)
        wq_r = wq.rearrange("(c p) e -> p c e", p=128)
        for dp in range(DC // 2):
            nc.gpsimd.dma_start(out=w_sb[:, 2 * dp:2 * dp + 2, :],
                                in_=wq_r[:, 2 * dp:2 * dp + 2, :])
            nc.gpsimd.dma_start(out=xT_sb[:, 2 * dp:2 * dp + 2, :],
                                in_=xT_r[:, 2 * dp:2 * dp + 2, :])
        bqk_sb = singles.tile([128, 2 * EC // 128], F32)
        nc.gpsimd.dma_start(out=bqk_sb, in_=bqk.rearrange("(c p) -> p c", p=128))
        bvb_sb = singles.tile([128, EC], F32)
        bv_b = bass.AP(tensor=bv.tensor, offset=bv.offset,
                       ap=[[0, 128]] + list(bv.ap))
        nc.gpsimd.dma_start(out=bvb_sb, in_=bv_b)
        mask_sb = singles.tile([128, 896], BDT)
        nc.gpsimd.dma_start(out=mask_sb, in_=mask)
        wo_sb = singles.tile([128, EC // 128, D], BDT)
        nc.gpsimd.dma_start(out=wo_sb, in_=wo.rearrange("(c p) e -> p c e", p=128))
        nc.gpsimd.load_library(library_config.attn)

        qkT_sb = acts.tile([128, 2 * EC // 128, T], BDT)
        vones_sb = acts.tile([128, TT, HPC, HS + 1], BDT)
        oT_sb = acts.tile([128, EC // 128, T], BDT)
        nc.vector.memset(vones_sb[:, :, :, HS:HS + 1], 1.0)

        def emit_qk(et, ts, ptag="s", pbufs=3):
            ps = psa.tile([128, 512], F32, tag=ptag, bufs=pbufs, name="psqk")
            for dc in range(DC):
                nc.tensor.matmul(
                    ps,
                    lhsT=w_sb[:, dc, et * 128:(et + 1) * 128],
                    rhs=xT_sb[:, dc, ts * 512:(ts + 1) * 512],
                    start=(dc == 0),
                    stop=(dc == DC - 1),
                )
            nc.vector.tensor_scalar_add(
                out=qkT_sb[:, et, ts * 512:(ts + 1) * 512],
                in0=ps,
                scalar1=bqk_sb[:, et:et + 1],
            )

        def emit_v(tt, ptag="s", pbufs=3):
            psv = psa.tile([128, EC], F32, tag=ptag, bufs=pbufs, name="psv")
            for dc in range(DC):
                nc.tensor.matmul(
                    psv,
                    lhsT=xT_sb[:, dc, tt * 128:(tt + 1) * 128],
                    rhs=w_sb[:, dc, 2 * EC:3 * EC],
                    start=(dc == 0),
                    stop=(dc == DC - 1),
                )
            nc.vector.tensor_add(
                out=vones_sb[:, tt, :, 0:HS],
                in0=psv.rearrange("p (h s) -> p h s", h=HPC),
                in1=bvb_sb.rearrange("p (h s) -> p h s", h=HPC),
            )

        def emit_attn(h, qs):
            pb = 64 * (h % 2)
            qT = qkT_sb[pb:pb + 64, h // 2, :]
            kT = qkT_sb[pb:pb + 64, 2 + h // 2, :]
            po = psa.tile([65, 512], F32, tag="o", bufs=2)
            nblk = (qs + 1) * 4

            def emit_pv(pT, quad):
                for (kb, off, q0, nq) in quad:
                    nc.tensor.matmul(
                        po[:, q0:512],
                        lhsT=vones_sb[:, kb, h, :],
                        rhs=pT[:, off:off + nq],
                        start=(kb == 0),
                        stop=(kb == nblk - 1),
                    )

            prev = None
            for quad in _quads(qs):
                qw = max(off + nq for (kb, off, q0, nq) in quad)
                if qw <= 512:
                    sps = psa.tile([128, 512], F32, tag="o", bufs=2, name="spsb")
                else:
                    sps = psa.tile([128, 1024], F32, tag="s", bufs=3, name="sps")
                pT = ptp.tile([128, 1024], BDT, tag="pT", name="pT")
                for (kb, off, q0, nq) in quad:
                    nc.tensor.matmul(
                        sps[:, off:off + nq],
                        lhsT=kT[:, kb * 128:(kb + 1) * 128],
                        rhs=qT[:, qs * 512 + q0:(qs + 1) * 512],
                        start=True,
                        stop=True,
                    )
                w = max(off + nq for (kb, off, q0, nq) in quad)
                nc.scalar.activation(out=pT[:, 0:w], in_=sps[:, 0:w],
                                     func=EXP, scale=SCALE)
                for (kb, off, q0, nq) in quad:
                    if kb >= qs * 4:
                        nc.vector.tensor_mul(
                            out=pT[:, off:off + 128],
                            in0=pT[:, off:off + 128],
                            in1=mask_sb[:, 384:512],
                        )
                if prev is not None:
                    emit_pv(*prev)
                prev = (pT, quad)
            emit_pv(*prev)

            rl = rlp.tile([1, 512], F32, tag="rl")
            nc.vector.reciprocal(out=rl, in_=po[64:65, :])
            rlb = rlp.tile([64, 512], F32, tag="rlb")
            nc.gpsimd.partition_broadcast(out_ap=rlb, in_ap=rl)
            nc.vector.tensor_mul(
                out=oT_sb[pb:pb + 64, h // 2, qs * 512:(qs + 1) * 512],
                in0=po[0:64, :],
                in1=rlb,
            )

        def emit_outproj(tt):
            outsb = obp.tile([128, 1024], F16, tag="ob", name="outsb")
            for half in range(2):
                pr = psa.tile([128, 512], F32, tag="s", bufs=3, name="pso")
                for ec in range(EC // 128):
                    nc.tensor.matmul(
                        pr,
                        lhsT=oT_sb[:, ec, tt * 128:(tt + 1) * 128],
                        rhs=wo_sb[:, ec, half * 512:(half + 1) * 512],
                        start=(ec == 0),
                        stop=(ec == EC // 128 - 1),
                    )
                if (tt + half) % 2 == 0:
                    nc.scalar.copy(out=outsb[:, half * 512:(half + 1) * 512], in_=pr)
                else:
                    nc.vector.tensor_copy(out=outsb[:, half * 512:(half + 1) * 512],
                                          in_=pr)
            nc.sync.dma_start(out=outp[tt * 128:(tt + 1) * 128, :], in_=outsb)

        pre_tags = ["s", "o", "s", "o", "s", "o", "s", "o"]
        for i, et in enumerate((0, 2, 1, 3)):
            emit_qk(et, 0, ptag=pre_tags[i], pbufs=3 if pre_tags[i] == "s" else 2)
        for i, tt in enumerate(range(4)):
            emit_v(tt, ptag=pre_tags[4 + i], pbufs=3 if pre_tags[4 + i] == "s" else 2)
        for qs in range(NQS):
            fillers = []
            if qs < NQS - 1:
                fillers += [lambda et=et: emit_qk(et, qs + 1) for et in (0, 2, 1, 3)]
                fillers += [lambda tt=tt: emit_v(tt) for tt in range(4 * qs + 4, 4 * qs + 8)]
            if qs >= 1:
                fillers += [lambda tt=tt: emit_outproj(tt) for tt in range(4 * (qs - 1), 4 * qs)]
            for h in range(HPC):
                emit_attn(h, qs)
                for f in fillers[(h * len(fillers)) // HPC:((h + 1) * len(fillers)) // HPC]:
                    f()
        for tt in range(4 * (NQS - 1), 4 * NQS):
            emit_outproj(tt)


def build_nc_bias():
    nc = bacc.Bacc("TRN2", target_bir_lowering=False, debug=False)
    xT = nc.dram_tensor("xT", (D, T), F32, kind="ExternalInput")
    wq = nc.dram_tensor("wq", (D, 3 * EC), F32, kind="ExternalInput")
    wo = nc.dram_tensor("wo", (EC, D), F32, kind="ExternalInput")
    bqk = nc.dram_tensor("bqk", (2 * EC,), F32, kind="ExternalInput")
    bv = nc.dram_tensor("bv", (EC,), F32, kind="ExternalInput")
    mask = nc.dram_tensor("mask", (128, 896), mybir.dt.bfloat16,
                          kind="ExternalInput")
    outp = nc.dram_tensor("outp", (T, D), F16, kind="ExternalOutput")
    with tile.TileContext(nc) as tc:
        _mha_tile_kernel_bias(tc, outp[:], xT[:], wq[:], wo[:], bqk[:], bv[:],
                              mask[:])
    nc.compile()
    return nc


def _host_mask_bias():
    import ml_dtypes
    x = np.arange(128)[:, None]
    j = np.arange(896)[None, :]
    return (j >= x + 384).astype(ml_dtypes.bfloat16)


def _kernel_bias(x, w_qkv, b_qkv, w_out, b_out):
    if "bias" not in _NC_CACHE:
        _NC_CACHE["bias"] = build_nc_bias()
    nc = _NC_CACHE["bias"]
    mask = _host_mask_bias()
    in_maps = []
    for c in range(NCORES):
        b, g = divmod(c, GROUPS)
        cs = slice(EC * g, EC * (g + 1))
        wq_c = np.ascontiguousarray(
            np.concatenate(
                [w_qkv[:, cs], w_qkv[:, D:][:, cs], w_qkv[:, 2 * D:][:, cs]],
                axis=1
            )
        )
        in_maps.append({
            "xT": np.ascontiguousarray(x[b].T),
            "wq": wq_c,
            "wo": np.ascontiguousarray(w_out[cs, :]),
            "bqk": np.ascontiguousarray(
                np.concatenate([b_qkv[cs], b_qkv[D:][cs]])
            ),
            "bv": np.ascontiguousarray(b_qkv[2 * D:][cs]),
            "mask": mask,
        })
    res = bass_utils.run_bass_kernel_spmd(
        nc, in_maps, core_ids=list(range(NCORES))
    )
    parts = [r["outp"].astype(np.float64) for r in res.results]
    out = np.stack([
        sum(parts[GROUPS * b:GROUPS * (b + 1)]) for b in range(B)
    ]).astype(np.float32)
    return out + b_out[None, None, :]


# revision 13
# speedup vs baseline: 1.0564x; 1.0564x over previous
# Multi-head causal attention (B=2, T=2048, D=1024, H=16, HS=64) on 8 TRN2 NeuronCores.
#
# Sharding: core c = (batch b = c//4, head-group g = c%4 -> heads 4g..4g+3).
# Host pre-transposes x (kernel input xT = x[b].T, cast f16) and slices
# w_qkv columns / w_out rows per core; each core computes a partial (T, D)
# output projection and the host sums the 4 partials per batch (+ b_out).
#
# On-device dataflow (fast path, biases == 0):
#   Q^T,K^T [hs, t] come out of the QKV projection (w stationary, x^T moving);
#   V is computed in natural [t, hs] layout with an extra ones-column.
#   Scores are built as S^T [k, t] blocks, exp'd on ACT (no max-subtraction:
#   scores bounded, fp32 exp safe), diag triangles masked on GpSimd.
#   PV runs in "o-orientation": stationary P^T blocks [128k x 128q], moving
#   vones [128k, 65] -> po [128 q-part, 65] accumulated over k-blocks (the
#   65th column is the softmax denominator l).  This streams only 65 columns
#   per k-block (vs 128 q-cols in the oT orientation), halving PV PE time.
#   Normalization: 1/l per-partition scalar (DVE reciprocal + tensor_scalar).
#   o [q, e] is then PE-transposed (identity rhs) to oT [e, t] feeding the
#   output projection as the stationary operand.
import math
import os
import sys

import numpy as np

for _p in ("/opt/trn_rl_repo",):
    if _p not in sys.path and os.path.isdir(_p):
        sys.path.insert(0, _p)

import concourse.bass as bass
import concourse.mybir as mybir
import concourse.tile as tile
from concourse import bacc
from concourse import bass_utils
from concourse import library_config

B, T, D = 2, 2048, 1024
H, HS = 16, 64
NCORES = 8
GROUPS = NCORES // B          # head-groups per batch = 4
HPC = H // GROUPS             # heads per core = 4
EC = HPC * HS                 # head-dim cols per section per core = 256
DC = D // 128                 # d-chunks = 8
TT = T // 128                 # t-tiles = 16
QS = 512                      # q-supertile
NQS = T // QS                 # 4
SCALE = 1.0 / math.sqrt(HS)

F32 = mybir.dt.float32
F16 = mybir.dt.float16
CDT = mybir.dt.float16        # compute dtype for matmul operands


def _quads(qs):
    # quad = list of (kb, col_off, q0, nq); diagonal blocks packed
    # contiguously so one exp covers only valid columns.
    qds = []
    for kq in range(qs * 2):
        qds.append([(kq * 2, 0, 0, 512), (kq * 2 + 1, 512, 0, 512)])
    d0 = qs * 4
    qds.append([(d0 + 0, 0, 0, 512), (d0 + 1, 512, 128, 384)])
    qds.append([(d0 + 2, 0, 256, 256), (d0 + 3, 256, 384, 128)])
    return qds


def _locate(kb, qb, qs):
    # (quad index, col offset) of score block (k-block kb, q-block qb) in the
    # round-qs quad tiles.
    if kb < 4 * qs:
        return kb // 2, (kb % 2) * 512 + 128 * qb
    j = kb - 4 * qs
    if j == 0:
        return 2 * qs, 128 * qb
    if j == 1:
        return 2 * qs, 512 + 128 * (qb - 1)
    if j == 2:
        return 2 * qs + 1, 128 * (qb - 2)
    return 2 * qs + 1, 256 + 128 * (qb - 3)


def _mha_fast_kernel(tc, outp, xT, wq, wo, mask, ident):
    nc = tc.nc
    EXP = mybir.ActivationFunctionType.Exp

    with (
        tc.tile_pool(name="singles", bufs=1) as singles,
        tc.tile_pool(name="acts", bufs=1) as acts,
        tc.tile_pool(name="pt", bufs=24) as ptp,
        tc.tile_pool(name="rl", bufs=4) as rlp,
        tc.tile_pool(name="ob", bufs=4) as obp,
        tc.tile_pool(name="psum", bufs=1, space="PSUM") as psa,
    ):
        xT_sb = singles.tile([128, DC, T], CDT)
        w_sb = singles.tile([128, DC, 3 * EC], CDT)
        wo_sb = singles.tile([128, EC // 128, D], CDT)
        mask_sb = singles.tile([128, 128], CDT)
        ident_sb = singles.tile([128, 128], CDT)

        xT_r = xT.rearrange("(c p) t -> p c t", p=128)
        wq_r = wq.rearrange("(c p) e -> p c e", p=128)

        # ---- input DMAs (HWDGE via sync; emission order = transfer order).
        # Slab-major x, q|k weight columns first, finely sliced at the start
        # so the ts=0 projection can begin ~3.8us in.
        for dp in range(4):
            nc.sync.dma_start(out=w_sb[:, 2 * dp:2 * dp + 2, 0:512],
                              in_=wq_r[:, 2 * dp:2 * dp + 2, 0:512])
            nc.sync.dma_start(out=xT_sb[:, 2 * dp:2 * dp + 2, 0:512],
                              in_=xT_r[:, 2 * dp:2 * dp + 2, 0:512])
        nc.sync.dma_start(out=w_sb[:, 0:4, 512:768], in_=wq_r[:, 0:4, 512:768])
        nc.sync.dma_start(out=w_sb[:, 4:8, 512:768], in_=wq_r[:, 4:8, 512:768])
        nc.sync.dma_start(out=mask_sb, in_=mask)
        nc.sync.dma_start(out=xT_sb[:, 0:4, 512:1024], in_=xT_r[:, 0:4, 512:1024])
        nc.sync.dma_start(out=xT_sb[:, 4:8, 512:1024], in_=xT_r[:, 4:8, 512:1024])
        nc.sync.dma_start(out=xT_sb[:, :, 1024:1536], in_=xT_r[:, :, 1024:1536])
        nc.sync.dma_start(out=ident_sb, in_=ident)
        nc.sync.dma_start(out=xT_sb[:, :, 1536:2048], in_=xT_r[:, :, 1536:2048])
        nc.sync.dma_start(out=wo_sb, in_=wo.rearrange("(c p) e -> p c e", p=128))
        nc.gpsimd.load_library(library_config.standard)

        qkT_sb = acts.tile([128, 2 * EC // 128, T], CDT)
        vones_sb = acts.tile([128, TT, HPC, HS + 1], CDT)
        o_sb = acts.tile([128, TT, EC], CDT)
        oT_sb = acts.tile([128, EC // 128, T], CDT)
        nc.vector.memset(vones_sb[:, :, :, HS:HS + 1], 1.0)

        # PSUM (8 banks): "s" scores 2x[128,1024]; "po" PV accumulators
        # 2x[128,288] (4 x 72-col chains per bank); "pj" 2x[128,512] shared by
        # projections / transposes / out-proj.
        def emit_qk_prologue():
            # ts=0 q,k projection, dc-major four-phase so each dc-pair's
            # matmuls run while the next x/w slices are still in flight.
            sA = psa.tile([128, 1024], F32, tag="s", bufs=2, name="sA")
            sB = psa.tile([128, 1024], F32, tag="s", bufs=2, name="sB")
            tgt = [sA[:, 0:512], sA[:, 512:1024], sB[:, 0:512], sB[:, 512:1024]]
            for phase in range(4):
                for et in range(4):
                    for dc in range(2 * phase, 2 * phase + 2):
                        nc.tensor.matmul(
                            tgt[et],
                            lhsT=w_sb[:, dc, et * 128:(et + 1) * 128],
                            rhs=xT_sb[:, dc, 0:512],
                            start=(dc == 0),
                            stop=(dc == DC - 1),
                        )
            nc.vector.tensor_copy(out=qkT_sb[:, 0:2, 0:512], in_=sA)
            nc.vector.tensor_copy(out=qkT_sb[:, 2:4, 0:512], in_=sB)

        def emit_qk(et, ts):
            ps = psa.tile([128, 512], F32, tag="pj", bufs=2, name="psqk")
            for dc in range(DC):
                nc.tensor.matmul(
                    ps,
                    lhsT=w_sb[:, dc, et * 128:(et + 1) * 128],
                    rhs=xT_sb[:, dc, ts * 512:(ts + 1) * 512],
                    start=(dc == 0),
                    stop=(dc == DC - 1),
                )
            nc.vector.tensor_copy(
                out=qkT_sb[:, et, ts * 512:(ts + 1) * 512], in_=ps)

        def emit_v2(tp):
            # V projection for tt pair (2tp, 2tp+1): two 8-dc chains packed in
            # one bank (single start on the first, single stop on the last).
            psv = psa.tile([128, 512], F32, tag="pj", bufs=2, name="psv")
            for i in range(2):
                ttc = 2 * tp + i
                for dc in range(DC):
                    nc.tensor.matmul(
                        psv[:, i * 256:(i + 1) * 256],
                        lhsT=xT_sb[:, dc, ttc * 128:(ttc + 1) * 128],
                        rhs=w_sb[:, dc, 2 * EC:3 * EC],
                        start=(i == 0 and dc == 0),
                        stop=(i == 1 and dc == DC - 1),
                    )
            nc.vector.tensor_copy(
                out=vones_sb[:, 2 * tp:2 * tp + 2, :, 0:HS],
                in_=psv.rearrange("p (t h s) -> p t h s", t=2, h=HPC),
            )

        def emit_quads(h, qs):
            # QK^T score blocks + exp + diag masking; returns the round's pT
            # tiles (one per quad).
            pb = 64 * (h % 2)
            qT = qkT_sb[pb:pb + 64, h // 2, :]
            kT = qkT_sb[pb:pb + 64, 2 + h // 2, :]
            pts = []
            for quad in _quads(qs):
                sps = psa.tile([128, 1024], F32, tag="s", bufs=2, name="sps")
                pT = ptp.tile([128, 1024], CDT, tag="pT", name="pT")
                for (kb, off, q0, nq) in quad:
                    nc.tensor.matmul(
                        sps[:, off:off + nq],
                        lhsT=kT[:, kb * 128:(kb + 1) * 128],
                        rhs=qT[:, qs * 512 + q0:(qs + 1) * 512],
                        start=True,
                        stop=True,
                    )
                w = max(off + nq for (kb, off, q0, nq) in quad)
                nc.scalar.activation(out=pT[:, 0:w], in_=sps[:, 0:w],
                                     func=EXP, scale=SCALE)
                for (kb, off, q0, nq) in quad:
                    if kb >= qs * 4:  # diagonal: mask leading 128-col triangle
                        nc.gpsimd.tensor_mul(
                            out=pT[:, off:off + 128],
                            in0=pT[:, off:off + 128],
                            in1=mask_sb,
                        )
                pts.append(pT)
            return pts

        def emit_chains(h, qs, pts, qbs=(0, 1, 2, 3)):
            # PV in o-orientation: po[q, 0:64] = sum_k P^T[k,q-blk] V[k,:],
            # po[q, 64] = l.  The 72-col chains share one PSUM bank; one
            # start (clears the bank) and one stop on the last matmul.
            po = psa.tile([128, 512], F32, tag="po", bufs=2, name="po")
            for i, qb in enumerate(qbs):
                Q = 4 * qs + qb
                for kb in range(Q + 1):
                    j, col = _locate(kb, qb, qs)
                    nc.tensor.matmul(
                        po[:, 72 * i:72 * i + 65],
                        lhsT=pts[j][:, col:col + 128],
                        rhs=vones_sb[:, kb, h, :],
                        start=(i == 0 and kb == 0),
                        stop=(i == len(qbs) - 1 and kb == Q),
                    )
            # epilogue: per-partition 1/l, fused into the PSUM->SBUF evac
            rl = rlp.tile([128, len(qbs)], F32, tag="rl")
            lcols = bass.AP(
                tensor=po.tensor, offset=po.offset + 64,
                ap=[list(po.ap[0]), [72, len(qbs)]],
            )
            nc.vector.reciprocal(out=rl, in_=lcols)
            for i, qb in enumerate(qbs):
                nc.vector.tensor_scalar_mul(
                    out=o_sb[:, 4 * qs + qb, h * 64:(h + 1) * 64],
                    in0=po[:, 72 * i:72 * i + 64],
                    scalar1=rl[:, i:i + 1],
                )

        def emit_transpose(tts, ec):
            # o [q, e] -> oT [e, t] for the given t-tiles, one ec block
            # (= head pair 2ec, 2ec+1); transposes packed in one bank.
            tp = psa.tile([128, 512], CDT, tag="pj", bufs=2, name="tp")
            for i, tt in enumerate(tts):
                nc.tensor.transpose(
                    tp[:, i * 128:(i + 1) * 128],
                    o_sb[:, tt, ec * 128:(ec + 1) * 128],
                    ident_sb,
                )
            nc.vector.tensor_copy(
                out=oT_sb[:, ec, tts[0] * 128:(tts[0] + len(tts)) * 128],
                in_=tp[:, 0:128 * len(tts)])

        def emit_outproj(tt, split_store=False):
            outsb = obp.tile([128, 1024], F16, tag="ob", name="outsb")
            for half in range(2):
                pr = psa.tile([128, 512], F32, tag="pj", bufs=2, name="pso")
                for ec in range(EC // 128):
                    nc.tensor.matmul(
                        pr,
                        lhsT=oT_sb[:, ec, tt * 128:(tt + 1) * 128],
                        rhs=wo_sb[:, ec, half * 512:(half + 1) * 512],
                        start=(ec == 0),
                        stop=(ec == EC // 128 - 1),
                    )
                nc.vector.tensor_copy(
                    out=outsb[:, half * 512:(half + 1) * 512], in_=pr)
                if split_store:
                    nc.sync.dma_start(
                        out=outp[tt * 128:(tt + 1) * 128,
                                 half * 512:(half + 1) * 512],
                        in_=outsb[:, half * 512:(half + 1) * 512])
            if not split_store:
                nc.sync.dma_start(out=outp[tt * 128:(tt + 1) * 128, :],
                                  in_=outsb)

        # ---- emission: flattened software pipeline over 16 head-rounds.
        # Score quads (+exp) run LAG head-rounds ahead of their PV chains so
        # ACT has lookahead in the exp-heavy late rounds.  Projections are
        # front-loaded; transposes/out-proj are deferred into the late
        # rounds, where the serial exp stream otherwise starves PE.
        LAG = 2
        emit_qk_prologue()
        emit_v2(0)
        emit_v2(1)

        fill = {i: [] for i in range(16)}

        def sched(items, lo, hi):
            n = hi - lo + 1
            for i, it in enumerate(items):
                fill[lo + (i * n) // len(items)].append(it)

        for qs in range(NQS - 1):
            items = [lambda et=et, qs=qs: emit_qk(et, qs + 1) for et in (0, 2, 1, 3)]
            vtp = [lambda tp=tp: emit_v2(tp) for tp in (2 * qs + 2, 2 * qs + 3)]
            if qs < 2:
                sched(items + vtp, 4 * qs, 4 * qs + 3)
            else:
                # V for the last round is PE filler for the exp-bound round 3
                sched(items, 8, 11)
                sched(vtp, 12, 13)
        for r in range(NQS - 1):
            items = [lambda ec=ec, r=r: emit_transpose(
                [4 * r + i for i in range(4)], ec) for ec in (0, 1)]
            items += [lambda tt=tt: emit_outproj(tt) for tt in range(4 * r, 4 * r + 4)]
            lo = 4 * r + 6
            sched(items, lo, min(lo + 3, 15))

        quads = {}
        for i in range(16):
            qs, h = divmod(i, 4)
            quads[i] = emit_quads(h, qs)
            if i - LAG >= 0:
                pqs, ph = divmod(i - LAG, 4)
                emit_chains(ph, pqs, quads.pop(i - LAG))
            for f in fill[i]:
                f()
        # drain: remaining chains, the very last head-round split by q-block
        # pair so the final out-projections overlap its PV.
        emit_chains(2, 3, quads[14])
        emit_chains(3, 3, quads[15], qbs=(0, 1))
        for ec in range(2):
            emit_transpose([12, 13], ec)
        emit_outproj(12)
        emit_outproj(13)
        emit_chains(3, 3, quads[15], qbs=(2, 3))
        for ec in range(2):
            emit_transpose([14, 15], ec)
        emit_outproj(14, split_store=True)
        emit_outproj(15, split_store=True)


def build_nc_fast():
    nc = bacc.Bacc("TRN2", target_bir_lowering=False, debug=False)
    xT = nc.dram_tensor("xT", (D, T), F16, kind="ExternalInput")
    wq = nc.dram_tensor("wq", (D, 3 * EC), F16, kind="ExternalInput")
    wo = nc.dram_tensor("wo", (EC, D), F16, kind="ExternalInput")
    mask = nc.dram_tensor("mask", (128, 128), F16, kind="ExternalInput")
    ident = nc.dram_tensor("ident", (128, 128), F16, kind="ExternalInput")
    outp = nc.dram_tensor("outp", (T, D), F16, kind="ExternalOutput")
    with tile.TileContext(nc) as tc:
        _mha_fast_kernel(tc, outp[:], xT[:], wq[:], wo[:], mask[:], ident[:])
    nc.compile()
    return nc


def make_in_maps_fast(x, w_qkv, w_out):
    import ml_dtypes
    f16 = np.float16
    i = np.arange(128)
    mask = (i[None, :] >= i[:, None]).astype(f16)     # keep q >= k
    ident = np.eye(128, dtype=f16)
    in_maps = []
    for c in range(NCORES):
        b, g = divmod(c, GROUPS)
        cs = slice(EC * g, EC * (g + 1))
        wq_c = np.ascontiguousarray(np.concatenate(
            [w_qkv[:, cs], w_qkv[:, D:][:, cs], w_qkv[:, 2 * D:][:, cs]],
            axis=1).astype(f16))
        in_maps.append({
            "xT": np.ascontiguousarray(x[b].T.astype(f16)),
            "wq": wq_c,
            "wo": np.ascontiguousarray(w_out[cs, :].astype(f16)),
            "mask": mask,
            "ident": ident,
        })
    return in_maps


_NC_CACHE = {}


def get_nc():
    if "fast" not in _NC_CACHE:
        _NC_CACHE["fast"] = build_nc_fast()
    return _NC_CACHE["fast"]


def run_on_hw(in_maps, **kwargs):
    nc = get_nc()
    return bass_utils.run_bass_kernel_spmd(
        nc, in_maps, core_ids=list(range(NCORES)), **kwargs
    )


def kernel(x, w_qkv, b_qkv, w_out, b_out):
    x = np.asarray(x, dtype=np.float32)
    w_qkv = np.asarray(w_qkv, dtype=np.float32)
    b_qkv = np.asarray(b_qkv, dtype=np.float32)
    w_out = np.asarray(w_out, dtype=np.float32)
    b_out = np.asarray(b_out, dtype=np.float32)

    if np.any(b_qkv):
        return _kernel_bias(x, w_qkv, b_qkv, w_out, b_out)

    in_maps = make_in_maps_fast(x, w_qkv, w_out)
    res = run_on_hw(in_maps)
    parts = [r["outp"].astype(np.float64) for r in res.results]
    out = np.stack([
        sum(parts[GROUPS * b:GROUPS * (b + 1)]) for b in range(B)
    ]).astype(np.float32)
    return out + b_out[None, None, :]


# ---------------------------------------------------------------------------
# Fallback path for nonzero b_qkv (not hit by the reference input
# distribution): the original bias-capable kernel.
# ---------------------------------------------------------------------------


def _mha_tile_kernel_bias(tc, outp, xT, wq, wo, bqk, bv, mask):
    nc = tc.nc
    EXP = mybir.ActivationFunctionType.Exp
    BDT = mybir.dt.bfloat16

    with (
        tc.tile_pool(name="singles", bufs=1) as singles,
        tc.tile_pool(name="acts", bufs=1) as acts,
        tc.tile_pool(name="pt", bufs=8) as ptp,
        tc.tile_pool(name="rl", bufs=6) as rlp,
        tc.tile_pool(name="ob", bufs=6) as obp,
        tc.tile_pool(name="psum", bufs=1, space="PSUM") as psa,
    ):
        xT_sb = singles.tile([128, DC, T], BDT)
        w_sb = singles.tile([128, DC, 3 * EC], BDT)
        xT_r = xT.rearrange("(c p) t -> p c t", p=128)
        wq_r = wq.rearrange("(c p) e -> p c e", p=128)
        for dp in range(DC // 2):
            nc.gpsimd.dma_start(out=w_sb[:, 2 * dp:2 * dp + 2, :],
                                in_=wq_r[:, 2 * dp:2 * dp + 2, :])
            nc.gpsimd.dma_start(out=xT_sb[:, 2 * dp:2 * dp + 2, :],
                                in_=xT_r[:, 2 * dp:2 * dp + 2, :])
        bqk_sb = singles.tile([128, 2 * EC // 128], F32)
        nc.gpsimd.dma_start(out=bqk_sb, in_=bqk.rearrange("(c p) -> p c", p=128))
        bvb_sb = singles.tile([128, EC], F32)
        bv_b = bass.AP(tensor=bv.tensor, offset=bv.offset,
                       ap=[[0, 128]] + list(bv.ap))
        nc.gpsimd.dma_start(out=bvb_sb, in_=bv_b)
        mask_sb = singles.tile([128, 896], BDT)
        nc.gpsimd.dma_start(out=mask_sb, in_=mask)
        wo_sb = singles.tile([128, EC // 128, D], BDT)
        nc.gpsimd.dma_start(out=wo_sb, in_=wo.rearrange("(c p) e -> p c e", p=128))
        nc.gpsimd.load_library(library_config.attn)

        qkT_sb = acts.tile([128, 2 * EC // 128, T], BDT)
        vones_sb = acts.tile([128, TT, HPC, HS + 1], BDT)
        oT_sb = acts.tile([128, EC // 128, T], BDT)
        nc.vector.memset(vones_sb[:, :, :, HS:HS + 1], 1.0)

        def emit_qk(et, ts, ptag="s", pbufs=3):
            ps = psa.tile([128, 512], F32, tag=ptag, bufs=pbufs, name="psqk")
            for dc in range(DC):
                nc.tensor.matmul(
                    ps,
                    lhsT=w_sb[:, dc, et * 128:(et + 1) * 128],
                    rhs=xT_sb[:, dc, ts * 512:(ts + 1) * 512],
                    start=(dc == 0),
                    stop=(dc == DC - 1),
                )
            nc.vector.tensor_scalar_add(
                out=qkT_sb[:, et, ts * 512:(ts + 1) * 512],
                in0=ps,
                scalar1=bqk_sb[:, et:et + 1],
            )

        def emit_v(tt, ptag="s", pbufs=3):
            psv = psa.tile([128, EC], F32, tag=ptag, bufs=pbufs, name="psv")
            for dc in range(DC):
                nc.tensor.matmul(
                    psv,
                    lhsT=xT_sb[:, dc, tt * 128:(tt + 1) * 128],
                    rhs=w_sb[:, dc, 2 * EC:3 * EC],
                    start=(dc == 0),
                    stop=(dc == DC - 1),
                )
            nc.vector.tensor_add(
                out=vones_sb[:, tt, :, 0:HS],
                in0=psv.rearrange("p (h s) -> p h s", h=HPC),
                in1=bvb_sb.rearrange("p (h s) -> p h s", h=HPC),
            )

        def emit_attn(h, qs):
            pb = 64 * (h % 2)
            qT = qkT_sb[pb:pb + 64, h // 2, :]
            kT = qkT_sb[pb:pb + 64, 2 + h // 2, :]
            po = psa.tile([65, 512], F32, tag="o", bufs=2)
            nblk = (qs + 1) * 4

            def emit_pv(pT, quad):
                for (kb, off, q0, nq) in quad:
                    nc.tensor.matmul(
                        po[:, q0:512],
                        lhsT=vones_sb[:, kb, h, :],
                        rhs=pT[:, off:off + nq],
                        start=(kb == 0),
                        stop=(kb == nblk - 1),
                    )

            prev = None
            for quad in _quads(qs):
                qw = max(off + nq for (kb, off, q0, nq) in quad)
                if qw <= 512:
                    sps = psa.tile([128, 512], F32, tag="o", bufs=2, name="spsb")
                else:
                    sps = psa.tile([128, 1024], F32, tag="s", bufs=3, name="sps")
                pT = ptp.tile([128, 1024], BDT, tag="pT", name="pT")
                for (kb, off, q0, nq) in quad:
                    nc.tensor.matmul(
                        sps[:, off:off + nq],
                        lhsT=kT[:, kb * 128:(kb + 1) * 128],
                        rhs=qT[:, qs * 512 + q0:(qs + 1) * 512],
                        start=True,
                        stop=True,
                    )
                w = max(off + nq for (kb, off, q0, nq) in quad)
                nc.scalar.activation(out=pT[:, 0:w], in_=sps[:, 0:w],
                                     func=EXP, scale=SCALE)
                for (kb, off, q0, nq) in quad:
                    if kb >= qs * 4:
                        nc.vector.tensor_mul(
                            out=pT[:, off:off + 128],
                            in0=pT[:, off:off + 128],
                            in1=mask_sb[:, 384:512],
                        )
                if prev is not None:
                    emit_pv(*prev)
                prev = (pT, quad)
            emit_pv(*prev)

            rl = rlp.tile([1, 512], F32, tag="rl")
            nc.vector.reciprocal(out=rl, in_=po[64:65, :])
            rlb = rlp.tile([64, 512], F32, tag="rlb")
            nc.gpsimd.partition_broadcast(out_ap=rlb, in_ap=rl)
            nc.vector.tensor_mul(
                out=oT_sb[pb:pb + 64, h // 2, qs * 512:(qs + 1) * 512],
                in0=po[0:64, :],
                in1=rlb,
            )

        def emit_outproj(tt):
            outsb = obp.tile([128, 1024], F16, tag="ob", name="outsb")
            for half in range(2):
                pr = psa.tile([128, 512], F32, tag="s", bufs=3, name="pso")
                for ec in range(EC // 128):
                    nc.tensor.matmul(
                        pr,
                        lhsT=oT_sb[:, ec, tt * 128:(tt + 1) * 128],
                        rhs=wo_sb[:, ec, half * 512:(half + 1) * 512],
                        start=(ec == 0),
                        stop=(ec == EC // 128 - 1),
                    )
                if (tt + half) % 2 == 0:
                    nc.scalar.copy(out=outsb[:, half * 512:(half + 1) * 512], in_=pr)
                else:
                    nc.vector.tensor_copy(out=outsb[:, half * 512:(half + 1) * 512],
                                          in_=pr)
            nc.sync.dma_start(out=outp[tt * 128:(tt + 1) * 128, :], in_=outsb)

        pre_tags = ["s", "o", "s", "o", "s", "o", "s", "o"]
        for i, et in enumerate((0, 2, 1, 3)):
            emit_qk(et, 0, ptag=pre_tags[i], pbufs=3 if pre_tags[i] == "s" else 2)
        for i, tt in enumerate(range(4)):
            emit_v(tt, ptag=pre_tags[4 + i], pbufs=3 if pre_tags[4 + i] == "s" else 2)
        for qs in range(NQS):
            fillers = []
            if qs < NQS - 1:
                fillers += [lambda et=et: emit_qk(et, qs + 1) for et in (0, 2, 1, 3)]
                fillers += [lambda tt=tt: emit_v(tt) for tt in range(4 * qs + 4, 4 * qs + 8)]
            if qs >= 1:
                fillers += [lambda tt=tt: emit_outproj(tt) for tt in range(4 * (qs - 1), 4 * qs)]
            for h in range(HPC):
                emit_attn(h, qs)
                for f in fillers[(h * len(fillers)) // HPC:((h + 1) * len(fillers)) // HPC]:
                    f()
        for tt in range(4 * (NQS - 1), 4 * NQS):
            emit_outproj(tt)


def build_nc_bias():
    nc = bacc.Bacc("TRN2", target_bir_lowering=False, debug=False)
    xT = nc.dram_tensor("xT", (D, T), F32, kind="ExternalInput")
    wq = nc.dram_tensor("wq", (D, 3 * EC), F32, kind="ExternalInput")
    wo = nc.dram_tensor("wo", (EC, D), F32, kind="ExternalInput")
    bqk = nc.dram_tensor("bqk", (2 * EC,), F32, kind="ExternalInput")
    bv = nc.dram_tensor("bv", (EC,), F32, kind="ExternalInput")
    mask = nc.dram_tensor("mask", (128, 896), mybir.dt.bfloat16,
                          kind="ExternalInput")
    outp = nc.dram_tensor("outp", (T, D), F16, kind="ExternalOutput")
    with tile.TileContext(nc) as tc:
        _mha_tile_kernel_bias(tc, outp[:], xT[:], wq[:], wo[:], bqk[:], bv[:],
                              mask[:])
    nc.compile()
    return nc


def _host_mask_bias():
    import ml_dtypes
    x = np.arange(128)[:, None]
    j = np.arange(896)[None, :]
    return (j >= x + 384).astype(ml_dtypes.bfloat16)


def _kernel_bias(x, w_qkv, b_qkv, w_out, b_out):
    if "bias" not in _NC_CACHE:
        _NC_CACHE["bias"] = build_nc_bias()
    nc = _NC_CACHE["bias"]
    mask = _host_mask_bias()
    in_maps = []
    for c in range(NCORES):
        b, g = divmod(c, GROUPS)
        cs = slice(EC * g, EC * (g + 1))
        wq_c = np.ascontiguousarray(
            np.concatenate(
                [w_qkv[:, cs], w_qkv[:, D:][:, cs], w_qkv[:, 2 * D:][:, cs]],
                axis=1
            )
        )
        in_maps.append({
            "xT": np.ascontiguousarray(x[b].T),
            "wq": wq_c,
            "wo": np.ascontiguousarray(w_out[cs, :]),
            "bqk": np.ascontiguousarray(
                np.concatenate([b_qkv[cs], b_qkv[D:][cs]])
            ),
            "bv": np.ascontiguousarray(b_qkv[2 * D:][cs]),
            "mask": mask,
        })
    res = bass_utils.run_bass_kernel_spmd(
        nc, in_maps, core_ids=list(range(NCORES))
    )
    parts = [r["outp"].astype(np.float64) for r in res.results]
    out = np.stack([
        sum(parts[GROUPS * b:GROUPS * (b + 1)]) for b in range(B)
    ]).astype(np.float32)
    return out + b_out[None, None, :]


# revision 18
# speedup vs baseline: 1.0882x; 1.0301x over previous
# Multi-head causal attention (B=2, T=2048, D=1024, H=16, HS=64) on 8 TRN2 NeuronCores.
#
# Sharding: core c = (batch b = c//4, head-group g = c%4 -> heads 4g..4g+3).
# Host pre-transposes x (kernel input xT = x[b].T, cast f16) and slices
# w_qkv columns / w_out rows per core; each core computes a partial (T, D)
# output projection and the host sums the 4 partials per batch (+ b_out).
#
# On-device dataflow (fast path, biases == 0):
#   Q^T,K^T [hs, t] come out of the QKV projection (w stationary, x^T moving);
#   V is computed in natural [t, hs] layout with an extra ones-column.
#   Scores are built as S^T [k, t] blocks, exp'd on ACT (no max-subtraction:
#   scores bounded, fp32 exp safe), diag triangles masked on GpSimd.
#   PV runs in "o-orientation": stationary P^T blocks [128k x 128q], moving
#   vones [128k, 65] -> po [128 q-part, 65] accumulated over k-blocks (the
#   65th column is the softmax denominator l).  This streams only 65 columns
#   per k-block (vs 128 q-cols in the oT orientation), halving PV PE time.
#   Normalization: 1/l per-partition scalar (DVE reciprocal + tensor_scalar).
#   o [q, e] is then PE-transposed (identity rhs) to oT [e, t] feeding the
#   output projection as the stationary operand.
import math
import os
import sys

import numpy as np

for _p in ("/opt/trn_rl_repo",):
    if _p not in sys.path and os.path.isdir(_p):
        sys.path.insert(0, _p)

import concourse.bass as bass
import concourse.mybir as mybir
import concourse.tile as tile
from concourse import bacc
from concourse import bass_utils
from concourse import library_config

B, T, D = 2, 2048, 1024
H, HS = 16, 64
NCORES = 8
GROUPS = NCORES // B          # head-groups per batch = 4
HPC = H // GROUPS             # heads per core = 4
EC = HPC * HS                 # head-dim cols per section per core = 256
DC = D // 128                 # d-chunks = 8
TT = T // 128                 # t-tiles = 16
QS = 512                      # q-supertile
NQS = T // QS                 # 4
SCALE = 1.0 / math.sqrt(HS)

F32 = mybir.dt.float32
F16 = mybir.dt.float16
CDT = mybir.dt.float16        # compute dtype for matmul operands


def _quads(qs):
    # quad = list of (kb, col_off, q0, nq); diagonal blocks packed
    # contiguously so one exp covers only valid columns.
    qds = []
    for kq in range(qs * 2):
        qds.append([(kq * 2, 0, 0, 512), (kq * 2 + 1, 512, 0, 512)])
    d0 = qs * 4
    qds.append([(d0 + 0, 0, 0, 512), (d0 + 1, 512, 128, 384)])
    qds.append([(d0 + 2, 0, 256, 256), (d0 + 3, 256, 384, 128)])
    return qds


def _locate(kb, qb, qs):
    # (quad index, col offset) of score block (k-block kb, q-block qb) in the
    # round-qs quad tiles.
    if kb < 4 * qs:
        return kb // 2, (kb % 2) * 512 + 128 * qb
    j = kb - 4 * qs
    if j == 0:
        return 2 * qs, 128 * qb
    if j == 1:
        return 2 * qs, 512 + 128 * (qb - 1)
    if j == 2:
        return 2 * qs + 1, 128 * (qb - 2)
    return 2 * qs + 1, 256 + 128 * (qb - 3)


def _mha_fast_kernel(tc, outp, xT, wq, wo, mask, ident):
    nc = tc.nc
    EXP = mybir.ActivationFunctionType.Exp

    with (
        tc.tile_pool(name="singles", bufs=1) as singles,
        tc.tile_pool(name="acts", bufs=1) as acts,
        tc.tile_pool(name="pt", bufs=24) as ptp,
        tc.tile_pool(name="rl", bufs=4) as rlp,
        tc.tile_pool(name="ob", bufs=4) as obp,
        tc.tile_pool(name="psum", bufs=1, space="PSUM") as psa,
    ):
        xT_sb = singles.tile([128, DC, T], CDT)
        w_sb = singles.tile([128, DC, 3 * EC], CDT)
        wo_sb = singles.tile([128, EC // 128, D], CDT)
        mask_sb = singles.tile([128, 128], CDT)
        ident_sb = singles.tile([128, 128], CDT)

        xT_r = xT.rearrange("(c p) t -> p c t", p=128)
        wq_r = wq.rearrange("(c p) e -> p c e", p=128)

        # ---- input DMAs (HWDGE via sync; emission order = transfer order).
        # Slab-major x, q|k weight columns first, finely sliced at the start
        # so the ts=0 projection can begin ~3us in.
        for dc in range(2):
            nc.sync.dma_start(out=w_sb[:, dc:dc + 1, 0:512],
                              in_=wq_r[:, dc:dc + 1, 0:512])
            nc.sync.dma_start(out=xT_sb[:, dc:dc + 1, 0:512],
                              in_=xT_r[:, dc:dc + 1, 0:512])
        for dp in range(1, 4):
            nc.sync.dma_start(out=w_sb[:, 2 * dp:2 * dp + 2, 0:512],
                              in_=wq_r[:, 2 * dp:2 * dp + 2, 0:512])
            nc.sync.dma_start(out=xT_sb[:, 2 * dp:2 * dp + 2, 0:512],
                              in_=xT_r[:, 2 * dp:2 * dp + 2, 0:512])
        nc.sync.dma_start(out=w_sb[:, 0:4, 512:768], in_=wq_r[:, 0:4, 512:768])
        nc.sync.dma_start(out=w_sb[:, 4:8, 512:768], in_=wq_r[:, 4:8, 512:768])
        nc.sync.dma_start(out=mask_sb, in_=mask)
        nc.sync.dma_start(out=xT_sb[:, 0:4, 512:1024], in_=xT_r[:, 0:4, 512:1024])
        nc.sync.dma_start(out=xT_sb[:, 4:8, 512:1024], in_=xT_r[:, 4:8, 512:1024])
        nc.sync.dma_start(out=xT_sb[:, :, 1024:1536], in_=xT_r[:, :, 1024:1536])
        nc.sync.dma_start(out=ident_sb, in_=ident)
        nc.sync.dma_start(out=xT_sb[:, :, 1536:2048], in_=xT_r[:, :, 1536:2048])
        nc.sync.dma_start(out=wo_sb, in_=wo.rearrange("(c p) e -> p c e", p=128))
        nc.gpsimd.load_library(library_config.standard)

        qkT_sb = acts.tile([128, 2 * EC // 128, T], CDT)
        vones_sb = acts.tile([128, TT, HPC, HS + 1], CDT)
        o_sb = acts.tile([128, TT, EC], CDT)
        oT_sb = acts.tile([128, EC // 128, T], CDT)
        nc.vector.memset(vones_sb[:, :, :, HS:HS + 1], 1.0)

        # PSUM (8 banks): "s" scores 2x[128,1024]; "po" PV accumulators
        # 2x[128,288] (4 x 72-col chains per bank); "pj" 2x[128,512] shared by
        # projections / transposes / out-proj.
        def emit_qk_prologue():
            # ts=0 q,k projection, dc-major four-phase so each dc-pair's
            # matmuls run while the next x/w slices are still in flight.
            sA = psa.tile([128, 1024], F32, tag="s", bufs=2, name="sA")
            sB = psa.tile([128, 1024], F32, tag="s", bufs=2, name="sB")
            tgt = [sA[:, 0:512], sA[:, 512:1024], sB[:, 0:512], sB[:, 512:1024]]
            for dcs in ([0], [1], [2, 3], [4, 5], [6, 7]):
                for et in range(4):
                    for dc in dcs:
                        nc.tensor.matmul(
                            tgt[et],
                            lhsT=w_sb[:, dc, et * 128:(et + 1) * 128],
                            rhs=xT_sb[:, dc, 0:512],
                            start=(dc == 0),
                            stop=(dc == DC - 1),
                        )
            nc.vector.tensor_copy(out=qkT_sb[:, 0:2, 0:512], in_=sA)
            nc.vector.tensor_copy(out=qkT_sb[:, 2:4, 0:512], in_=sB)

        def emit_qk(et, ts):
            ps = psa.tile([128, 512], F32, tag="pj", bufs=2, name="psqk")
            for dc in range(DC):
                nc.tensor.matmul(
                    ps,
                    lhsT=w_sb[:, dc, et * 128:(et + 1) * 128],
                    rhs=xT_sb[:, dc, ts * 512:(ts + 1) * 512],
                    start=(dc == 0),
                    stop=(dc == DC - 1),
                )
            nc.vector.tensor_copy(
                out=qkT_sb[:, et, ts * 512:(ts + 1) * 512], in_=ps)

        def emit_v2(tp):
            # V projection for tt pair (2tp, 2tp+1): two 8-dc chains packed in
            # one bank (single start on the first, single stop on the last).
            psv = psa.tile([128, 512], F32, tag="pj", bufs=2, name="psv")
            for i in range(2):
                ttc = 2 * tp + i
                for dc in range(DC):
                    nc.tensor.matmul(
                        psv[:, i * 256:(i + 1) * 256],
                        lhsT=xT_sb[:, dc, ttc * 128:(ttc + 1) * 128],
                        rhs=w_sb[:, dc, 2 * EC:3 * EC],
                        start=(i == 0 and dc == 0),
                        stop=(i == 1 and dc == DC - 1),
                    )
            nc.vector.tensor_copy(
                out=vones_sb[:, 2 * tp:2 * tp + 2, :, 0:HS],
                in_=psv.rearrange("p (t h s) -> p t h s", t=2, h=HPC),
            )

        def emit_quads(h, qs):
            # QK^T score blocks + exp + diag masking; returns the round's pT
            # tiles (one per quad).
            pb = 64 * (h % 2)
            qT = qkT_sb[pb:pb + 64, h // 2, :]
            kT = qkT_sb[pb:pb + 64, 2 + h // 2, :]
            pts = []
            for quad in _quads(qs):
                sps = psa.tile([128, 1024], F32, tag="s", bufs=2, name="sps")
                pT = ptp.tile([128, 1024], CDT, tag="pT", name="pT")
                for (kb, off, q0, nq) in quad:
                    nc.tensor.matmul(
                        sps[:, off:off + nq],
                        lhsT=kT[:, kb * 128:(kb + 1) * 128],
                        rhs=qT[:, qs * 512 + q0:(qs + 1) * 512],
                        start=True,
                        stop=True,
                    )
                w = max(off + nq for (kb, off, q0, nq) in quad)
                nc.scalar.activation(out=pT[:, 0:w], in_=sps[:, 0:w],
                                     func=EXP, scale=SCALE)
                for (kb, off, q0, nq) in quad:
                    if kb >= qs * 4:  # diagonal: mask leading 128-col triangle
                        nc.gpsimd.tensor_mul(
                            out=pT[:, off:off + 128],
                            in0=pT[:, off:off + 128],
                            in1=mask_sb,
                        )
                pts.append(pT)
            return pts

        def emit_chains(h, qs, pts, qbs=(0, 1, 2, 3), scalar_evac=False):
            # PV in o-orientation: po[q, 0:64] = sum_k P^T[k,q-blk] V[k,:],
            # po[q, 64] = l.  The 72-col chains share one PSUM bank; one
            # start (clears the bank) and one stop on the last matmul.
            po = psa.tile([128, 512], F32, tag="po", bufs=2, name="po")
            for i, qb in enumerate(qbs):
                Q = 4 * qs + qb
                for kb in range(Q + 1):
                    j, col = _locate(kb, qb, qs)
                    nc.tensor.matmul(
                        po[:, 72 * i:72 * i + 65],
                        lhsT=pts[j][:, col:col + 128],
                        rhs=vones_sb[:, kb, h, :],
                        start=(i == 0 and kb == 0),
                        stop=(i == len(qbs) - 1 and kb == Q),
                    )
            # epilogue: per-partition 1/l, fused into the PSUM->SBUF evac
            rl = rlp.tile([128, len(qbs)], F32, tag="rl")
            lcols = bass.AP(
                tensor=po.tensor, offset=po.offset + 64,
                ap=[list(po.ap[0]), [72, len(qbs)]],
            )
            nc.vector.reciprocal(out=rl, in_=lcols)
            for i, qb in enumerate(qbs):
                if scalar_evac:
                    nc.scalar.mul(
                        out=o_sb[:, 4 * qs + qb, h * 64:(h + 1) * 64],
                        in_=po[:, 72 * i:72 * i + 64],
                        mul=rl[:, i:i + 1],
                    )
                else:
                    nc.vector.tensor_scalar_mul(
                        out=o_sb[:, 4 * qs + qb, h * 64:(h + 1) * 64],
                        in0=po[:, 72 * i:72 * i + 64],
                        scalar1=rl[:, i:i + 1],
                    )

        def emit_transpose(tts, ec, scalar_evac=False):
            # o [q, e] -> oT [e, t] for the given t-tiles, one ec block
            # (= head pair 2ec, 2ec+1); transposes packed in one bank.
            tp = psa.tile([128, 512], CDT, tag="pj", bufs=2, name="tp")
            for i, tt in enumerate(tts):
                nc.tensor.transpose(
                    tp[:, i * 128:(i + 1) * 128],
                    o_sb[:, tt, ec * 128:(ec + 1) * 128],
                    ident_sb,
                )
            evac = nc.scalar.copy if scalar_evac else (
                lambda out, in_: nc.vector.tensor_copy(out=out, in_=in_))
            evac(
                out=oT_sb[:, ec, tts[0] * 128:(tts[0] + len(tts)) * 128],
                in_=tp[:, 0:128 * len(tts)])

        def emit_outproj(tt, split_store=False, scalar_evac=False):
            outsb = obp.tile([128, 1024], F16, tag="ob", name="outsb")
            for half in range(2):
                pr = psa.tile([128, 512], F32, tag="pj", bufs=2, name="pso")
                for ec in range(EC // 128):
                    nc.tensor.matmul(
                        pr,
                        lhsT=oT_sb[:, ec, tt * 128:(tt + 1) * 128],
                        rhs=wo_sb[:, ec, half * 512:(half + 1) * 512],
                        start=(ec == 0),
                        stop=(ec == EC // 128 - 1),
                    )
                if scalar_evac and half == 0:
                    nc.scalar.copy(out=outsb[:, 0:512], in_=pr)
                else:
                    nc.vector.tensor_copy(
                        out=outsb[:, half * 512:(half + 1) * 512], in_=pr)
                if split_store:
                    nc.sync.dma_start(
                        out=outp[tt * 128:(tt + 1) * 128,
                                 half * 512:(half + 1) * 512],
                        in_=outsb[:, half * 512:(half + 1) * 512])
            if not split_store:
                nc.sync.dma_start(out=outp[tt * 128:(tt + 1) * 128, :],
                                  in_=outsb)

        # ---- emission: flattened software pipeline over 16 head-rounds.
        # Score quads (+exp) run LAG head-rounds ahead of their PV chains so
        # ACT has lookahead in the exp-heavy late rounds.  Projections are
        # front-loaded; transposes/out-proj are deferred into the late
        # rounds, where the serial exp stream otherwise starves PE.
        LAG = 2
        emit_qk_prologue()
        emit_v2(0)
        emit_v2(1)

        fill = {i: [] for i in range(16)}

        def sched(items, lo, hi):
            n = hi - lo + 1
            for i, it in enumerate(items):
                fill[lo + (i * n) // len(items)].append(it)

        for qs in range(NQS - 1):
            items = [lambda et=et, qs=qs: emit_qk(et, qs + 1) for et in (0, 2, 1, 3)]
            vtp = [lambda tp=tp: emit_v2(tp) for tp in (2 * qs + 2, 2 * qs + 3)]
            if qs < 2:
                sched(items + vtp, 4 * qs, 4 * qs + 3)
            else:
                # V for the last round is PE filler for the exp-bound round 3
                sched(items, 8, 11)
                sched(vtp, 12, 13)
        for r in range(NQS - 1):
            items = [lambda ec=ec, r=r: emit_transpose(
                [4 * r + i for i in range(4)], ec) for ec in (0, 1)]
            items += [lambda tt=tt: emit_outproj(tt) for tt in range(4 * r, 4 * r + 4)]
            lo = 4 * r + 6
            sched(items, lo, min(lo + 3, 15))

        quads = {}
        for i in range(16):
            qs, h = divmod(i, 4)
            quads[i] = emit_quads(h, qs)
            if i - LAG >= 0:
                pqs, ph = divmod(i - LAG, 4)
                emit_chains(ph, pqs, quads.pop(i - LAG))
            for f in fill[i]:
                f()
        # drain: remaining chains, the very last head-round split by q-block
        # so the final out-projections overlap its PV; evacuations shift to
        # the (now idle) scalar engine to unclog DVE.
        emit_chains(2, 3, quads[14])
        emit_chains(3, 3, quads[15], qbs=(0, 1), scalar_evac=True)
        for ec in range(2):
            emit_transpose([12, 13], ec, scalar_evac=(ec == 0))
        emit_outproj(12, scalar_evac=True)
        emit_outproj(13)
        emit_chains(3, 3, quads[15], qbs=(2,), scalar_evac=True)
        emit_chains(3, 3, quads[15], qbs=(3,))
        for ec in range(2):
            emit_transpose([14, 15], ec, scalar_evac=(ec == 0))
        emit_outproj(14, split_store=True, scalar_evac=True)
        emit_outproj(15, split_store=True, scalar_evac=True)


def build_nc_fast():
    nc = bacc.Bacc("TRN2", target_bir_lowering=False, debug=False)
    xT = nc.dram_tensor("xT", (D, T), F16, kind="ExternalInput")
    wq = nc.dram_tensor("wq", (D, 3 * EC), F16, kind="ExternalInput")
    wo = nc.dram_tensor("wo", (EC, D), F16, kind="ExternalInput")
    mask = nc.dram_tensor("mask", (128, 128), F16, kind="ExternalInput")
    ident = nc.dram_tensor("ident", (128, 128), F16, kind="ExternalInput")
    outp = nc.dram_tensor("outp", (T, D), F16, kind="ExternalOutput")
    with tile.TileContext(nc) as tc:
        _mha_fast_kernel(tc, outp[:], xT[:], wq[:], wo[:], mask[:], ident[:])
    nc.compile()
    return nc


def make_in_maps_fast(x, w_qkv, w_out):
    import ml_dtypes
    f16 = np.float16
    i = np.arange(128)
    mask = (i[None, :] >= i[:, None]).astype(f16)     # keep q >= k
    ident = np.eye(128, dtype=f16)
    in_maps = []
    for c in range(NCORES):
        b, g = divmod(c, GROUPS)
        cs = slice(EC * g, EC * (g + 1))
        wq_c = np.ascontiguousarray(np.concatenate(
            [w_qkv[:, cs], w_qkv[:, D:][:, cs], w_qkv[:, 2 * D:][:, cs]],
            axis=1).astype(f16))
        in_maps.append({
            "xT": np.ascontiguousarray(x[b].T.astype(f16)),
            "wq": wq_c,
            "wo": np.ascontiguousarray(w_out[cs, :].astype(f16)),
            "mask": mask,
            "ident": ident,
        })
    return in_maps


_NC_CACHE = {}


def get_nc():
    if "fast" not in _NC_CACHE:
        _NC_CACHE["fast"] = build_nc_fast()
    return _NC_CACHE["fast"]


def run_on_hw(in_maps, **kwargs):
    nc = get_nc()
    return bass_utils.run_bass_kernel_spmd(
        nc, in_maps, core_ids=list(range(NCORES)), **kwargs
    )


def kernel(x, w_qkv, b_qkv, w_out, b_out):
    x = np.asarray(x, dtype=np.float32)
    w_qkv = np.asarray(w_qkv, dtype=np.float32)
    b_qkv = np.asarray(b_qkv, dtype=np.float32)
    w_out = np.asarray(w_out, dtype=np.float32)
    b_out = np.asarray(b_out, dtype=np.float32)

    if np.any(b_qkv):
        return _kernel_bias(x, w_qkv, b_qkv, w_out, b_out)

    in_maps = make_in_maps_fast(x, w_qkv, w_out)
    res = run_on_hw(in_maps)
    parts = [r["outp"].astype(np.float64) for r in res.results]
    out = np.stack([
        sum(parts[GROUPS * b:GROUPS * (b + 1)]) for b in range(B)
    ]).astype(np.float32)
    return out + b_out[None, None, :]


# ---------------------------------------------------------------------------
# Fallback path for nonzero b_qkv (not hit by the reference input
# distribution): the original bias-capable kernel.
# ---------------------------------------------------------------------------


def _mha_tile_kernel_bias(tc, outp, xT, wq, wo, bqk, bv, mask):
    nc = tc.nc
    EXP = mybir.ActivationFunctionType.Exp
    BDT = mybir.dt.bfloat16

    with (
        tc.tile_pool(name="singles", bufs=1) as singles,
        tc.tile_pool(name="acts", bufs=1) as acts,
        tc.tile_pool(name="pt", bufs=8) as ptp,
        tc.tile_pool(name="rl", bufs=6) as rlp,
        tc.tile_pool(name="ob", bufs=6) as obp,
        tc.tile_pool(name="psum", bufs=1, space="PSUM") as psa,
    ):
        xT_sb = singles.tile([128, DC, T], BDT)
        w_sb = singles.tile([128, DC, 3 * EC], BDT)
        xT_r = xT.rearrange("(c p) t -> p c t", p=128)
        wq_r = wq.rearrange("(c p) e -> p c e", p=128)
        for dp in range(DC // 2):
            nc.gpsimd.dma_start(out=w_sb[:, 2 * dp:2 * dp + 2, :],
                                in_=wq_r[:, 2 * dp:2 * dp + 2, :])
            nc.gpsimd.dma_start(out=xT_sb[:, 2 * dp:2 * dp + 2, :],
                                in_=xT_r[:, 2 * dp:2 * dp + 2, :])
        bqk_sb = singles.tile([128, 2 * EC // 128], F32)
        nc.gpsimd.dma_start(out=bqk_sb, in_=bqk.rearrange("(c p) -> p c", p=128))
        bvb_sb = singles.tile([128, EC], F32)
        bv_b = bass.AP(tensor=bv.tensor, offset=bv.offset,
                       ap=[[0, 128]] + list(bv.ap))
        nc.gpsimd.dma_start(out=bvb_sb, in_=bv_b)
        mask_sb = singles.tile([128, 896], BDT)
        nc.gpsimd.dma_start(out=mask_sb, in_=mask)
        wo_sb = singles.tile([128, EC // 128, D], BDT)
        nc.gpsimd.dma_start(out=wo_sb, in_=wo.rearrange("(c p) e -> p c e", p=128))
        nc.gpsimd.load_library(library_config.attn)

        qkT_sb = acts.tile([128, 2 * EC // 128, T], BDT)
        vones_sb = acts.tile([128, TT, HPC, HS + 1], BDT)
        oT_sb = acts.tile([128, EC // 128, T], BDT)
        nc.vector.memset(vones_sb[:, :, :, HS:HS + 1], 1.0)

        def emit_qk(et, ts, ptag="s", pbufs=3):
            ps = psa.tile([128, 512], F32, tag=ptag, bufs=pbufs, name="psqk")
            for dc in range(DC):
                nc.tensor.matmul(
                    ps,
                    lhsT=w_sb[:, dc, et * 128:(et + 1) * 128],
                    rhs=xT_sb[:, dc, ts * 512:(ts + 1) * 512],
                    start=(dc == 0),
                    stop=(dc == DC - 1),
                )
            nc.vector.tensor_scalar_add(
                out=qkT_sb[:, et, ts * 512:(ts + 1) * 512],
                in0=ps,
                scalar1=bqk_sb[:, et:et + 1],
            )

        def emit_v(tt, ptag="s", pbufs=3):
            psv = psa.tile([128, EC], F32, tag=ptag, bufs=pbufs, name="psv")
            for dc in range(DC):
                nc.tensor.matmul(
                    psv,
                    lhsT=xT_sb[:, dc, tt * 128:(tt + 1) * 128],
                    rhs=w_sb[:, dc, 2 * EC:3 * EC],
                    start=(dc == 0),
                    stop=(dc == DC - 1),
                )
            nc.vector.tensor_add(
                out=vones_sb[:, tt, :, 0:HS],
                in0=psv.rearrange("p (h s) -> p h s", h=HPC),
                in1=bvb_sb.rearrange("p (h s) -> p h s", h=HPC),
            )

        def emit_attn(h, qs):
            pb = 64 * (h % 2)
            qT = qkT_sb[pb:pb + 64, h // 2, :]
            kT = qkT_sb[pb:pb + 64, 2 + h // 2, :]
            po = psa.tile([65, 512], F32, tag="o", bufs=2)
            nblk = (qs + 1) * 4

            def emit_pv(pT, quad):
                for (kb, off, q0, nq) in quad:
                    nc.tensor.matmul(
                        po[:, q0:512],
                        lhsT=vones_sb[:, kb, h, :],
                        rhs=pT[:, off:off + nq],
                        start=(kb == 0),
                        stop=(kb == nblk - 1),
                    )

            prev = None
            for quad in _quads(qs):
                qw = max(off + nq for (kb, off, q0, nq) in quad)
                if qw <= 512:
                    sps = psa.tile([128, 512], F32, tag="o", bufs=2, name="spsb")
                else:
                    sps = psa.tile([128, 1024], F32, tag="s", bufs=3, name="sps")
                pT = ptp.tile([128, 1024], BDT, tag="pT", name="pT")
                for (kb, off, q0, nq) in quad:
                    nc.tensor.matmul(
                        sps[:, off:off + nq],
                        lhsT=kT[:, kb * 128:(kb + 1) * 128],
                        rhs=qT[:, qs * 512 + q0:(qs + 1) * 512],
                        start=True,
                        stop=True,
                    )
                w = max(off + nq for (kb, off, q0, nq) in quad)
                nc.scalar.activation(out=pT[:, 0:w], in_=sps[:, 0:w],
                                     func=EXP, scale=SCALE)
                for (kb, off, q0, nq) in quad:
                    if kb >= qs * 4:
                        nc.vector.tensor_mul(
                            out=pT[:, off:off + 128],
                            in0=pT[:, off:off + 128],
                            in1=mask_sb[:, 384:512],
                        )
                if prev is not None:
                    emit_pv(*prev)
                prev = (pT, quad)
            emit_pv(*prev)

            rl = rlp.tile([1, 512], F32, tag="rl")
            nc.vector.reciprocal(out=rl, in_=po[64:65, :])
            rlb = rlp.tile([64, 512], F32, tag="rlb")
            nc.gpsimd.partition_broadcast(out_ap=rlb, in_ap=rl)
            nc.vector.tensor_mul(
                out=oT_sb[pb:pb + 64, h // 2, qs * 512:(qs + 1) * 512],
                in0=po[0:64, :],
                in1=rlb,
            )

        def emit_outproj(tt):
            outsb = obp.tile([128, 1024], F16, tag="ob", name="outsb")
            for half in range(2):
                pr = psa.tile([128, 512], F32, tag="s", bufs=3, name="pso")
                for ec in range(EC // 128):
                    nc.tensor.matmul(
                        pr,
                        lhsT=oT_sb[:, ec, tt * 128:(tt + 1) * 128],
                        rhs=wo_sb[:, ec, half * 512:(half + 1) * 512],
                        start=(ec == 0),
                        stop=(ec == EC // 128 - 1),
                    )
                if (tt + half) % 2 == 0:
                    nc.scalar.copy(out=outsb[:, half * 512:(half + 1) * 512], in_=pr)
                else:
                    nc.vector.tensor_copy(out=outsb[:, half * 512:(half + 1) * 512],
                                          in_=pr)
            nc.sync.dma_start(out=outp[tt * 128:(tt + 1) * 128, :], in_=outsb)

        pre_tags = ["s", "o", "s", "o", "s", "o", "s", "o"]
        for i, et in enumerate((0, 2, 1, 3)):
            emit_qk(et, 0, ptag=pre_tags[i], pbufs=3 if pre_tags[i] == "s" else 2)
        for i, tt in enumerate(range(4)):
            emit_v(tt, ptag=pre_tags[4 + i], pbufs=3 if pre_tags[4 + i] == "s" else 2)
        for qs in range(NQS):
            fillers = []
            if qs < NQS - 1:
                fillers += [lambda et=et: emit_qk(et, qs + 1) for et in (0, 2, 1, 3)]
                fillers += [lambda tt=tt: emit_v(tt) for tt in range(4 * qs + 4, 4 * qs + 8)]
            if qs >= 1:
                fillers += [lambda tt=tt: emit_outproj(tt) for tt in range(4 * (qs - 1), 4 * qs)]
            for h in range(HPC):
                emit_attn(h, qs)
                for f in fillers[(h * len(fillers)) // HPC:((h + 1) * len(fillers)) // HPC]:
                    f()
        for tt in range(4 * (NQS - 1), 4 * NQS):
            emit_outproj(tt)


def build_nc_bias():
    nc = bacc.Bacc("TRN2", target_bir_lowering=False, debug=False)
    xT = nc.dram_tensor("xT", (D, T), F32, kind="ExternalInput")
    wq = nc.dram_tensor("wq", (D, 3 * EC), F32, kind="ExternalInput")
    wo = nc.dram_tensor("wo", (EC, D), F32, kind="ExternalInput")
    bqk = nc.dram_tensor("bqk", (2 * EC,), F32, kind="ExternalInput")
    bv = nc.dram_tensor("bv", (EC,), F32, kind="ExternalInput")
    mask = nc.dram_tensor("mask", (128, 896), mybir.dt.bfloat16,
                          kind="ExternalInput")
    outp = nc.dram_tensor("outp", (T, D), F16, kind="ExternalOutput")
    with tile.TileContext(nc) as tc:
        _mha_tile_kernel_bias(tc, outp[:], xT[:], wq[:], wo[:], bqk[:], bv[:],
                              mask[:])
    nc.compile()
    return nc


def _host_mask_bias():
    import ml_dtypes
    x = np.arange(128)[:, None]
    j = np.arange(896)[None, :]
    return (j >= x + 384).astype(ml_dtypes.bfloat16)


def _kernel_bias(x, w_qkv, b_qkv, w_out, b_out):
    if "bias" not in _NC_CACHE:
        _NC_CACHE["bias"] = build_nc_bias()
    nc = _NC_CACHE["bias"]
    mask = _host_mask_bias()
    in_maps = []
    for c in range(NCORES):
        b, g = divmod(c, GROUPS)
        cs = slice(EC * g, EC * (g + 1))
        wq_c = np.ascontiguousarray(
            np.concatenate(
                [w_qkv[:, cs], w_qkv[:, D:][:, cs], w_qkv[:, 2 * D:][:, cs]],
                axis=1
            )
        )
        in_maps.append({
            "xT": np.ascontiguousarray(x[b].T),
            "wq": wq_c,
            "wo": np.ascontiguousarray(w_out[cs, :]),
            "bqk": np.ascontiguousarray(
                np.concatenate([b_qkv[cs], b_qkv[D:][cs]])
            ),
            "bv": np.ascontiguousarray(b_qkv[2 * D:][cs]),
            "mask": mask,
        })
    res = bass_utils.run_bass_kernel_spmd(
        nc, in_maps, core_ids=list(range(NCORES))
    )
    parts = [r["outp"].astype(np.float64) for r in res.results]
    out = np.stack([
        sum(parts[GROUPS * b:GROUPS * (b + 1)]) for b in range(B)
    ]).astype(np.float32)
    return out + b_out[None, None, :]
